# revision 36
# baseline (speedup 1.0000x reference)
"""PillarVFE on 8 trn2 NeuronCores — v6: fp16 matmuls + Act/DVE chain-pair
PSUM drain, plane outputs, epilogue on host.

Math: per pillar p, point n with raw r=(x,y,z,w):
  out[p,o] = relu( max( max_n (r_n . A)[o] - Q_p[o],  C_p[o] ) )
where A[4,64] folds W + BN scale, Q_p folds the pillar-constant part
(center offsets + cluster mean) minus the BN bias, and C_p is the
candidate from masked points: c0 if npts<32 else -inf.  The device
computes partial maxes of S_p[o] = max_n (r_n . A)[o]; the cheap
elementwise epilogue (plane fold, half fold, -Q, max C, relu,
unpermute) runs on host.

Device structure: pillars sorted by npts desc, 10 slots x 512 pillars
per core; slot i runs B=ceil(maxN_i/2) point-pair matmuls (partition =
2x64 channels, free = 512 pillars) into 2-bank PSUM tiles (ring of 4 =
all 8 banks).  PSUM tiles drain in OFFSET PAIRS: Act copy-casts pair
p's tile A to fp16 SBUF (one 1024-col op), and DVE folds that cast
with pair p+1's tile B in one mixed-dtype tensor_max -> 2 fp16 planes;
the one-pair offset means the DVE op's cast input is long since ready,
so the only live dependency is its own psum tile.  4 banks retire per
(1112ns Act + 1192ns DVE) running concurrently; PE, Act and DVE are
all ~balanced (~27us each per core).  Chain seeds / odd tails drain
via DVE copy/reduce or Act cast chosen by a static load balancer.
Planes collect in a per-slot out tile -> HBM; host max-folds the few
planes per pillar.  All T inputs prefetch at kernel start; the
framework's unused const-tile memsets are suppressed (they gate the
init barrier on the slow-booting GPSIMD).  Empirical constraints:
TensorTensor reads at most one PSUM operand; GPSIMD can't touch PSUM
or run TensorTensor; no cross-partition ops (lane-locked engines);
strided Act writes are 5x slow; fp16 TT gets the 2x DVE mode, reduce
does not; matmul out <= 512 free columns (one PSUM bank).
"""

import sys

import numpy as np

sys.path.insert(0, "/opt/trn_rl_repo")

VX, VY = 0.16, 0.16
X_OFF = VX / 2 + 0.0
Y_OFF = VY / 2 + (-39.68)
BN_EPS = 1e-3

P_FULL = 40000
N_PTS = 32
C_OUT = 64
N_CORES = 8
N_SLOTS = 10
TILE_P = 512
P_PAD = N_CORES * N_SLOTS * TILE_P  # 40960

_CACHE = {}


def _plan_slots(sched):
    """Plane layout per slot + tail drain choice.  Returns per-slot
    (n_planes, n_pairs2, tail) with tail in {None,'sv','sa','r2v','r2a',
    'r3av','r3va'}."""
    est_v, est_a = 0.0, 0.0
    plan = []
    for maxN in sched:
        B = (maxN + 1) // 2
        n_pairs2 = B // 4
        rem = B - 4 * n_pairs2
        # the seed (first B tile of the offset chain) can be drained by
        # either engine; remaining pair work is fixed (casts on Act,
        # folds on DVE)
        seed = None
        if n_pairs2:
            est_a += n_pairs2 * 1112.0
            est_v += (n_pairs2 - 1) * 1192.0
            if est_v + 1192.0 <= est_a + 1112.0:
                seed = "v"
                est_v += 1192.0
            else:
                seed = "a"
                est_a += 1112.0
        n_planes = 2 * n_pairs2 + (2 if n_pairs2 else 0)
        tail = None
        if rem == 1:
            if est_v + 690.0 <= est_a + 570.0:
                tail, dv, da, pl = "sv", 690.0, 0.0, 1
            else:
                tail, dv, da, pl = "sa", 0.0, 570.0, 1
        elif rem == 2:
            if est_v + 1223.0 <= est_a + 1112.0:
                tail, dv, da, pl = "r2v", 1223.0, 0.0, 1
            else:
                tail, dv, da, pl = "r2a", 0.0, 1112.0, 2
        elif rem == 3:
            if max(est_v + 1223.0, est_a + 570.0) <= max(
                est_v + 690.0, est_a + 1112.0
            ):
                tail, dv, da, pl = "r3va", 1223.0, 570.0, 2
            else:
                tail, dv, da, pl = "r3av", 690.0, 1112.0, 3
        if tail is not None:
            est_v += dv
            est_a += da
            n_planes += pl
        plan.append((n_planes, n_pairs2, tail, seed))
    _plan_slots.est = (est_v, est_a)
    return plan


def _build_nc(sched):
    from contextlib import ExitStack

    from concourse import bass, tile
    from concourse import mybir

    f32 = mybir.dt.float32
    f16 = mybir.dt.float16
    # Skip the framework's const-tile memsets (unused by this kernel:
    # activation Copy with float bias reads no const APs).  They run on
    # the slow-booting GPSIMD engine and gate the init barrier ~1.5us.
    _orig_memset = bass.BassGpSimd.memset
    bass.BassGpSimd.memset = lambda self, ap, constant: None
    try:
        nc = bass.Bass()
    finally:
        bass.BassGpSimd.memset = _orig_memset

    plan = _plan_slots(sched)

    T_ds = []
    for i, maxN in enumerate(sched):
        G = (maxN + 7) // 8
        T_ds.append(
            nc.dram_tensor(f"T{i}", [32 * G, TILE_P], f16, kind="ExternalInput")
        )
    S_d = nc.dram_tensor("S", [128, 4, 128], f16, kind="ExternalInput")
    O_ds = [
        nc.dram_tensor(f"O{i}", [128, pl[0], TILE_P], f16, kind="ExternalOutput")
        for i, pl in enumerate(plan)
    ]

    with tile.TileContext(nc) as tc, ExitStack() as ctx:
        stat = ctx.enter_context(tc.tile_pool(name="stat", bufs=1))
        upool = ctx.enter_context(tc.tile_pool(name="upool", bufs=6))
        opool = ctx.enter_context(tc.tile_pool(name="opool", bufs=3))
        psum = ctx.enter_context(
            tc.tile_pool(name="ps", bufs=4, space=bass.MemorySpace.PSUM)
        )

        # prefetch stationaries + ALL slot inputs up front
        s_sb = stat.tile([128, 4, 128], f16)
        nc.sync.dma_start(s_sb[:], S_d[:])


        t_sbs = []
        for i, maxN in enumerate(sched):
            G = (maxN + 7) // 8
            t_sb = stat.tile([32 * G, TILE_P], f16, name=f"t{i}")
            nc.sync.dma_start(t_sb[:], T_ds[i][:])
            t_sbs.append(t_sb)

        for i, maxN in enumerate(sched):
            G = (maxN + 7) // 8
            n_planes, n_pairs2, tail, seed = plan[i]
            t_sb = t_sbs[i]

            pairs = [
                (w, g) for w in range(4) for g in range(G) if 8 * g + 2 * w < maxN
            ]
            B = (maxN + 1) // 2
            assert len(pairs) == B, (i, maxN, pairs)

            def mm2(pt, bank, j):
                w, g = pairs[j]
                nc.tensor.matmul(
                    pt[:, bank, :],
                    s_sb[32 * g : 32 * g + 32, w, :],
                    t_sb[32 * g : 32 * g + 32, :],
                    start=True,
                    stop=True,
                    tile_position=(32 * g, 0),
                )

            out_sb = opool.tile([128, n_planes, TILE_P], f16, name="o")
            idx = 0
            j = 0
            # offset-paired b=2 pairs: DVE folds pair p's B tile with the
            # cast of pair p-1's A tile (already long done -> no handoff
            # stall); first B is a plain copy, last cast goes straight to
            # the out tile.
            prev_u = None
            for p in range(n_pairs2):
                # fill the DVE-consumed tile first so its fold (which
                # only waits on these matmuls) starts as early as
                # possible; the Act cast of pa feeds the NEXT pair
                pb = psum.tile([128, 2, TILE_P], f32, name="pt")
                mm2(pb, 0, j); mm2(pb, 1, j + 1)
                pa = psum.tile([128, 2, TILE_P], f32, name="pt")
                mm2(pa, 0, j + 2); mm2(pa, 1, j + 3)
                j += 4
                if p == n_pairs2 - 1:
                    nc.scalar.copy(out_sb[:, idx : idx + 2, :], pa[:])
                    idx += 2
                else:
                    u = upool.tile([128, 2, TILE_P], f16, name="u")
                    nc.scalar.copy(u[:], pa[:])
                if prev_u is None:
                    if seed == "a":
                        nc.scalar.copy(out_sb[:, idx : idx + 2, :], pb[:])
                    else:
                        nc.vector.tensor_copy(out_sb[:, idx : idx + 2, :], pb[:])
                else:
                    nc.vector.tensor_max(
                        out_sb[:, idx : idx + 2, :], prev_u[:], pb[:]
                    )
                idx += 2
                if p != n_pairs2 - 1:
                    prev_u = u
            if tail in ("sv", "sa"):
                pt = psum.tile([128, 2, TILE_P], f32, name="pt")
                mm2(pt, 0, j)
                j += 1
                if tail == "sv":
                    nc.vector.tensor_copy(out_sb[:, idx, :], pt[:, 0, :])
                else:
                    nc.scalar.copy(out_sb[:, idx, :], pt[:, 0, :])
                idx += 1
            elif tail in ("r2v", "r2a"):
                pt = psum.tile([128, 2, TILE_P], f32, name="pt")
                mm2(pt, 0, j); mm2(pt, 1, j + 1)
                j += 2
                if tail == "r2v":
                    nc.vector.tensor_reduce(
                        out_sb[:, idx, :],
                        pt[:].transpose([0, 2, 1]),
                        axis=mybir.AxisListType.X,
                        op=mybir.AluOpType.max,
                    )
                    idx += 1
                else:
                    nc.scalar.copy(out_sb[:, idx : idx + 2, :], pt[:])
                    idx += 2
            elif tail in ("r3av", "r3va"):
                pa = psum.tile([128, 2, TILE_P], f32, name="pt")
                mm2(pa, 0, j); mm2(pa, 1, j + 1)
                pb = psum.tile([128, 2, TILE_P], f32, name="pt")
                mm2(pb, 0, j + 2)
                j += 3
                if tail == "r3va":
                    nc.vector.tensor_reduce(
                        out_sb[:, idx, :],
                        pa[:].transpose([0, 2, 1]),
                        axis=mybir.AxisListType.X,
                        op=mybir.AluOpType.max,
                    )
                    nc.scalar.copy(out_sb[:, idx + 1, :], pb[:, 0, :])
                else:
                    nc.scalar.copy(out_sb[:, idx : idx + 2, :], pa[:])
                    nc.vector.tensor_copy(out_sb[:, idx + 2, :], pb[:, 0, :])
                idx += 2 if tail == "r3va" else 3
            assert idx == n_planes and j == B, (i, idx, n_planes, j, B)
            nc.sync.dma_start(O_ds[i][:], out_sb[:])

    nc.finalize()
    import bass_rust

    # walrus codegen allows at most 1 sync wait per instruction
    bass_rust.generate_event_semaphores(nc)
    return nc


def _plan(voxels, W, gamma, beta, running_mean, running_var,
          voxel_num_points, voxel_coords):
    V = voxels.astype(np.float64)
    npts = voxel_num_points.astype(np.int64)
    coords = voxel_coords.astype(np.float64)
    W64 = W.astype(np.float64)
    s = gamma.astype(np.float64) / np.sqrt(running_var.astype(np.float64) + BN_EPS)
    c0 = beta.astype(np.float64) - running_mean.astype(np.float64) * s

    A = np.stack([
        s * (W64[:, 0] + W64[:, 4] + W64[:, 7]),
        s * (W64[:, 1] + W64[:, 5] + W64[:, 8]),
        s * (W64[:, 2] + W64[:, 6]),
        s * W64[:, 3],
    ], axis=0)  # [4,64]

    cx = coords[:, 3] * VX + X_OFF
    cy = coords[:, 2] * VY + Y_OFF
    m = V[:, :, :3].sum(axis=1) / npts[:, None]
    q = (cx[:, None] * (s * (W64[:, 0] + W64[:, 7]))[None, :]
         + cy[:, None] * (s * (W64[:, 1] + W64[:, 8]))[None, :]
         + m[:, 0:1] * (s * W64[:, 4])[None, :]
         + m[:, 1:2] * (s * W64[:, 5])[None, :]
         + m[:, 2:3] * (s * W64[:, 6])[None, :])
    Q = (q - c0[None, :]).astype(np.float32)                    # [P,64]
    C = np.where((npts < N_PTS)[:, None], c0[None, :], -1e30).astype(np.float32)

    Vmod = voxels.astype(np.float16).copy()
    invalid = np.arange(N_PTS)[None, :] >= npts[:, None]
    Vmod[invalid] = np.broadcast_to(Vmod[:, 0:1, :], Vmod.shape)[invalid]

    pad = P_PAD - P_FULL
    Vp = np.concatenate([Vmod, np.zeros((pad, N_PTS, 4), np.float16)], axis=0)
    Qp = np.concatenate([Q, np.zeros((pad, C_OUT), np.float32)], axis=0)
    Cp = np.concatenate([C, np.zeros((pad, C_OUT), np.float32)], axis=0)
    np_pad = np.concatenate([npts, np.ones(pad, np.int64)])

    order = np.argsort(-np_pad, kind="stable")
    ns = np_pad[order]
    sched = tuple(int(ns[N_CORES * TILE_P * i]) for i in range(N_SLOTS))

    # stationaries: S[32g+4j+c, w, m] = A[c, m%64] if j == 2w + m//64
    A16 = A.astype(np.float16)
    S_small = np.zeros((32, 4, 128), np.float16)
    for w in range(4):
        for half in range(2):
            jj = 2 * w + half
            S_small[4 * jj : 4 * jj + 4, w, 64 * half : 64 * half + 64] = A16
    S = np.tile(S_small, (4, 1, 1))  # [128,4,128]

    Vs = Vp[order]
    in_maps = []
    for k in range(N_CORES):
        mp = {"S": S}
        for i, maxN in enumerate(sched):
            G = (maxN + 7) // 8
            c = N_CORES * i + k
            sl = slice(TILE_P * c, TILE_P * (c + 1))
            mp[f"T{i}"] = np.ascontiguousarray(
                Vs[sl][:, : 8 * G, :].transpose(1, 2, 0).reshape(32 * G, TILE_P)
            )
        in_maps.append(mp)
    return in_maps, sched, order, Qp[order], Cp[order]


def _gather(results, sched, order, Qs, Cs):
    smax = np.empty((P_PAD, C_OUT), np.float32)
    for k in range(N_CORES):
        for i in range(N_SLOTS):
            Ok = results[k][f"O{i}"]  # [128, n_planes, 512] fp16
            pm = Ok.max(axis=1)       # [128, 512]
            fold = np.maximum(pm[:C_OUT, :], pm[C_OUT:, :]).astype(np.float32)
            c = N_CORES * i + k
            smax[TILE_P * c : TILE_P * (c + 1)] = fold.T
    out_sorted = np.maximum(np.maximum(smax - Qs, Cs), 0.0)
    out_full = np.empty_like(out_sorted)
    out_full[order] = out_sorted
    return np.ascontiguousarray(out_full[:P_FULL])


def kernel(**inputs):
    from concourse.bass_utils import run_bass_kernel_spmd

    in_maps, sched, order, Qs, Cs = _plan(**inputs)
    if sched not in _CACHE:
        _CACHE[sched] = _build_nc(sched)
    res = run_bass_kernel_spmd(_CACHE[sched], in_maps, list(range(N_CORES)))
    return _gather(res.results, sched, order, Qs, Cs)


# revision 39
# speedup vs baseline: 1.0651x; 1.0651x over previous
"""PillarVFE on 8 trn2 NeuronCores — v6: fp16 matmuls + Act/DVE chain-pair
PSUM drain, plane outputs, epilogue on host.

Math: per pillar p, point n with raw r=(x,y,z,w):
  out[p,o] = relu( max( max_n (r_n . A)[o] - Q_p[o],  C_p[o] ) )
where A[4,64] folds W + BN scale, Q_p folds the pillar-constant part
(center offsets + cluster mean) minus the BN bias, and C_p is the
candidate from masked points: c0 if npts<32 else -inf.  The device
computes partial maxes of S_p[o] = max_n (r_n . A)[o]; the cheap
elementwise epilogue (plane fold, half fold, -Q, max C, relu,
unpermute) runs on host.

Device structure: pillars sorted by npts desc, 10 slots x 512 pillars
per core; slot i runs B=ceil(maxN_i/2) point-pair matmuls (partition =
2x64 channels, free = 512 pillars) into 2-bank PSUM tiles (ring of 4 =
all 8 banks).  PSUM tiles drain in OFFSET PAIRS: Act copy-casts pair
p's tile A to fp16 SBUF (one 1024-col op), and DVE folds that cast
with pair p+1's tile B in one mixed-dtype tensor_max -> 2 fp16 planes;
the one-pair offset means the DVE op's cast input is long since ready,
so the only live dependency is its own psum tile.  4 banks retire per
(1112ns Act + 1192ns DVE) running concurrently; PE, Act and DVE are
all ~balanced (~27us each per core).  Chain seeds / odd tails drain
via DVE copy/reduce or Act cast chosen by a static load balancer.
Planes collect in a per-slot out tile -> HBM; host max-folds the few
planes per pillar.  All T inputs prefetch at kernel start; the
framework's unused const-tile memsets are suppressed (they gate the
init barrier on the slow-booting GPSIMD).  Empirical constraints:
TensorTensor reads at most one PSUM operand; GPSIMD can't touch PSUM
or run TensorTensor; no cross-partition ops (lane-locked engines);
strided Act writes are 5x slow; fp16 TT gets the 2x DVE mode, reduce
does not; matmul out <= 512 free columns (one PSUM bank).
"""

import sys

import numpy as np

sys.path.insert(0, "/opt/trn_rl_repo")

VX, VY = 0.16, 0.16
X_OFF = VX / 2 + 0.0
Y_OFF = VY / 2 + (-39.68)
BN_EPS = 1e-3

P_FULL = 40000
N_PTS = 32
C_OUT = 64
N_CORES = 8
N_SLOTS = 10
TILE_P = 512
P_PAD = N_CORES * N_SLOTS * TILE_P  # 40960

_CACHE = {}


def _plan_slots(sched):
    """Plane layout per slot + tail drain choice.  Returns per-slot
    (n_planes, n_pairs2, tail) with tail in {None,'sv','sa','r2v','r2a',
    'r3av','r3va'}."""
    est_v, est_a = 0.0, 0.0
    plan = []
    for maxN in sched:
        B = (maxN + 1) // 2
        n_pairs2 = B // 4
        rem = B - 4 * n_pairs2
        # offset-2 chain: casts on Act, folds/seeds on DVE; the last two
        # casts and first two B tiles ship as unfolded planes
        seed = "v"
        est_a += n_pairs2 * 1112.0
        est_v += n_pairs2 * 1192.0
        if n_pairs2 >= 2:
            n_planes = 2 * n_pairs2 + 4
        elif n_pairs2 == 1:
            n_planes = 4
        else:
            n_planes = 0
        tail = None
        if rem == 1:
            if est_v + 690.0 <= est_a + 570.0:
                tail, dv, da, pl = "sv", 690.0, 0.0, 1
            else:
                tail, dv, da, pl = "sa", 0.0, 570.0, 1
        elif rem == 2:
            if est_v + 1223.0 <= est_a + 1112.0:
                tail, dv, da, pl = "r2v", 1223.0, 0.0, 1
            else:
                tail, dv, da, pl = "r2a", 0.0, 1112.0, 2
        elif rem == 3:
            if max(est_v + 1223.0, est_a + 570.0) <= max(
                est_v + 690.0, est_a + 1112.0
            ):
                tail, dv, da, pl = "r3va", 1223.0, 570.0, 2
            else:
                tail, dv, da, pl = "r3av", 690.0, 1112.0, 3
        if tail is not None:
            est_v += dv
            est_a += da
            n_planes += pl
        plan.append((n_planes, n_pairs2, tail, seed))
    _plan_slots.est = (est_v, est_a)
    return plan


def _build_nc(sched):
    from contextlib import ExitStack

    from concourse import bass, tile
    from concourse import mybir

    f32 = mybir.dt.float32
    f16 = mybir.dt.float16
    # Skip the framework's const-tile memsets (unused by this kernel:
    # activation Copy with float bias reads no const APs).  They run on
    # the slow-booting GPSIMD engine and gate the init barrier ~1.5us.
    _orig_memset = bass.BassGpSimd.memset
    bass.BassGpSimd.memset = lambda self, ap, constant: None
    try:
        nc = bass.Bass()
    finally:
        bass.BassGpSimd.memset = _orig_memset

    plan = _plan_slots(sched)

    T_ds = []
    for i, maxN in enumerate(sched):
        G = (maxN + 7) // 8
        T_ds.append(
            nc.dram_tensor(f"T{i}", [32 * G, TILE_P], f16, kind="ExternalInput")
        )
    S_d = nc.dram_tensor("S", [128, 4, 128], f16, kind="ExternalInput")
    O_ds = [
        nc.dram_tensor(f"O{i}", [128, pl[0], TILE_P], f16, kind="ExternalOutput")
        for i, pl in enumerate(plan)
    ]

    with tile.TileContext(nc) as tc, ExitStack() as ctx:
        stat = ctx.enter_context(tc.tile_pool(name="stat", bufs=1))
        upool = ctx.enter_context(tc.tile_pool(name="upool", bufs=6))
        opool = ctx.enter_context(tc.tile_pool(name="opool", bufs=3))
        psum = ctx.enter_context(
            tc.tile_pool(name="ps", bufs=4, space=bass.MemorySpace.PSUM)
        )

        # prefetch stationaries + ALL slot inputs up front
        s_sb = stat.tile([128, 4, 128], f16)
        nc.sync.dma_start(s_sb[:], S_d[:])


        t_sbs = []
        for i, maxN in enumerate(sched):
            G = (maxN + 7) // 8
            t_sb = stat.tile([32 * G, TILE_P], f16, name=f"t{i}")
            nc.sync.dma_start(t_sb[:], T_ds[i][:])
            t_sbs.append(t_sb)

        for i, maxN in enumerate(sched):
            G = (maxN + 7) // 8
            n_planes, n_pairs2, tail, seed = plan[i]
            t_sb = t_sbs[i]

            pairs = [
                (w, g) for w in range(4) for g in range(G) if 8 * g + 2 * w < maxN
            ]
            B = (maxN + 1) // 2
            assert len(pairs) == B, (i, maxN, pairs)

            def mm2(pt, bank, j):
                w, g = pairs[j]
                nc.tensor.matmul(
                    pt[:, bank, :],
                    s_sb[32 * g : 32 * g + 32, w, :],
                    t_sb[32 * g : 32 * g + 32, :],
                    start=True,
                    stop=True,
                    tile_position=(32 * g, 0),
                )

            out_sb = opool.tile([128, n_planes, TILE_P], f16, name="o")
            idx = 0
            j = 0
            # offset-2 pairs: DVE folds pair p's B tile with the cast of
            # pair p-2's A tile (two pair-cadences of slack -> the fold
            # never waits on Act); first two B tiles are plain copies,
            # last two casts go straight to the out tile.
            uq = []
            for p in range(n_pairs2):
                pa = psum.tile([128, 2, TILE_P], f32, name="pt")
                mm2(pa, 0, j); mm2(pa, 1, j + 1)
                pb = psum.tile([128, 2, TILE_P], f32, name="pt")
                mm2(pb, 0, j + 2); mm2(pb, 1, j + 3)
                j += 4
                if p >= n_pairs2 - 2:
                    nc.scalar.copy(out_sb[:, idx : idx + 2, :], pa[:])
                    idx += 2
                else:
                    u = upool.tile([128, 2, TILE_P], f16, name="u")
                    nc.scalar.copy(u[:], pa[:])
                    uq.append(u)
                if p <= 1:
                    nc.vector.tensor_copy(out_sb[:, idx : idx + 2, :], pb[:])
                else:
                    nc.vector.tensor_max(
                        out_sb[:, idx : idx + 2, :], uq.pop(0)[:], pb[:]
                    )
                idx += 2
            if tail in ("sv", "sa"):
                pt = psum.tile([128, 2, TILE_P], f32, name="pt")
                mm2(pt, 0, j)
                j += 1
                if tail == "sv":
                    nc.vector.tensor_copy(out_sb[:, idx, :], pt[:, 0, :])
                else:
                    nc.scalar.copy(out_sb[:, idx, :], pt[:, 0, :])
                idx += 1
            elif tail in ("r2v", "r2a"):
                pt = psum.tile([128, 2, TILE_P], f32, name="pt")
                mm2(pt, 0, j); mm2(pt, 1, j + 1)
                j += 2
                if tail == "r2v":
                    nc.vector.tensor_reduce(
                        out_sb[:, idx, :],
                        pt[:].transpose([0, 2, 1]),
                        axis=mybir.AxisListType.X,
                        op=mybir.AluOpType.max,
                    )
                    idx += 1
                else:
                    nc.scalar.copy(out_sb[:, idx : idx + 2, :], pt[:])
                    idx += 2
            elif tail in ("r3av", "r3va"):
                pa = psum.tile([128, 2, TILE_P], f32, name="pt")
                mm2(pa, 0, j); mm2(pa, 1, j + 1)
                pb = psum.tile([128, 2, TILE_P], f32, name="pt")
                mm2(pb, 0, j + 2)
                j += 3
                if tail == "r3va":
                    nc.vector.tensor_reduce(
                        out_sb[:, idx, :],
                        pa[:].transpose([0, 2, 1]),
                        axis=mybir.AxisListType.X,
                        op=mybir.AluOpType.max,
                    )
                    nc.scalar.copy(out_sb[:, idx + 1, :], pb[:, 0, :])
                else:
                    nc.scalar.copy(out_sb[:, idx : idx + 2, :], pa[:])
                    nc.vector.tensor_copy(out_sb[:, idx + 2, :], pb[:, 0, :])
                idx += 2 if tail == "r3va" else 3
            assert idx == n_planes and j == B, (i, idx, n_planes, j, B)
            nc.sync.dma_start(O_ds[i][:], out_sb[:])

    nc.finalize()
    import bass_rust

    # walrus codegen allows at most 1 sync wait per instruction
    bass_rust.generate_event_semaphores(nc)
    return nc


def _plan(voxels, W, gamma, beta, running_mean, running_var,
          voxel_num_points, voxel_coords):
    V = voxels.astype(np.float64)
    npts = voxel_num_points.astype(np.int64)
    coords = voxel_coords.astype(np.float64)
    W64 = W.astype(np.float64)
    s = gamma.astype(np.float64) / np.sqrt(running_var.astype(np.float64) + BN_EPS)
    c0 = beta.astype(np.float64) - running_mean.astype(np.float64) * s

    A = np.stack([
        s * (W64[:, 0] + W64[:, 4] + W64[:, 7]),
        s * (W64[:, 1] + W64[:, 5] + W64[:, 8]),
        s * (W64[:, 2] + W64[:, 6]),
        s * W64[:, 3],
    ], axis=0)  # [4,64]

    cx = coords[:, 3] * VX + X_OFF
    cy = coords[:, 2] * VY + Y_OFF
    m = V[:, :, :3].sum(axis=1) / npts[:, None]
    q = (cx[:, None] * (s * (W64[:, 0] + W64[:, 7]))[None, :]
         + cy[:, None] * (s * (W64[:, 1] + W64[:, 8]))[None, :]
         + m[:, 0:1] * (s * W64[:, 4])[None, :]
         + m[:, 1:2] * (s * W64[:, 5])[None, :]
         + m[:, 2:3] * (s * W64[:, 6])[None, :])
    Q = (q - c0[None, :]).astype(np.float32)                    # [P,64]
    C = np.where((npts < N_PTS)[:, None], c0[None, :], -1e30).astype(np.float32)

    Vmod = voxels.astype(np.float16).copy()
    invalid = np.arange(N_PTS)[None, :] >= npts[:, None]
    Vmod[invalid] = np.broadcast_to(Vmod[:, 0:1, :], Vmod.shape)[invalid]

    pad = P_PAD - P_FULL
    Vp = np.concatenate([Vmod, np.zeros((pad, N_PTS, 4), np.float16)], axis=0)
    Qp = np.concatenate([Q, np.zeros((pad, C_OUT), np.float32)], axis=0)
    Cp = np.concatenate([C, np.zeros((pad, C_OUT), np.float32)], axis=0)
    np_pad = np.concatenate([npts, np.ones(pad, np.int64)])

    order = np.argsort(-np_pad, kind="stable")
    ns = np_pad[order]
    sched = tuple(int(ns[N_CORES * TILE_P * i]) for i in range(N_SLOTS))

    # stationaries: S[32g+4j+c, w, m] = A[c, m%64] if j == 2w + m//64
    A16 = A.astype(np.float16)
    S_small = np.zeros((32, 4, 128), np.float16)
    for w in range(4):
        for half in range(2):
            jj = 2 * w + half
            S_small[4 * jj : 4 * jj + 4, w, 64 * half : 64 * half + 64] = A16
    S = np.tile(S_small, (4, 1, 1))  # [128,4,128]

    Vs = Vp[order]
    in_maps = []
    for k in range(N_CORES):
        mp = {"S": S}
        for i, maxN in enumerate(sched):
            G = (maxN + 7) // 8
            c = N_CORES * i + k
            sl = slice(TILE_P * c, TILE_P * (c + 1))
            mp[f"T{i}"] = np.ascontiguousarray(
                Vs[sl][:, : 8 * G, :].transpose(1, 2, 0).reshape(32 * G, TILE_P)
            )
        in_maps.append(mp)
    return in_maps, sched, order, Qp[order], Cp[order]


def _gather(results, sched, order, Qs, Cs):
    smax = np.empty((P_PAD, C_OUT), np.float32)
    for k in range(N_CORES):
        for i in range(N_SLOTS):
            Ok = results[k][f"O{i}"]  # [128, n_planes, 512] fp16
            pm = Ok.max(axis=1)       # [128, 512]
            fold = np.maximum(pm[:C_OUT, :], pm[C_OUT:, :]).astype(np.float32)
            c = N_CORES * i + k
            smax[TILE_P * c : TILE_P * (c + 1)] = fold.T
    out_sorted = np.maximum(np.maximum(smax - Qs, Cs), 0.0)
    out_full = np.empty_like(out_sorted)
    out_full[order] = out_sorted
    return np.ascontiguousarray(out_full[:P_FULL])


def kernel(**inputs):
    from concourse.bass_utils import run_bass_kernel_spmd

    in_maps, sched, order, Qs, Cs = _plan(**inputs)
    if sched not in _CACHE:
        _CACHE[sched] = _build_nc(sched)
    res = run_bass_kernel_spmd(_CACHE[sched], in_maps, list(range(N_CORES)))
    return _gather(res.results, sched, order, Qs, Cs)


# revision 41
# speedup vs baseline: 1.1653x; 1.0941x over previous
"""PillarVFE on 8 trn2 NeuronCores — v6: fp16 matmuls + Act/DVE chain-pair
PSUM drain, plane outputs, epilogue on host.

Math: per pillar p, point n with raw r=(x,y,z,w):
  out[p,o] = relu( max( max_n (r_n . A)[o] - Q_p[o],  C_p[o] ) )
where A[4,64] folds W + BN scale, Q_p folds the pillar-constant part
(center offsets + cluster mean) minus the BN bias, and C_p is the
candidate from masked points: c0 if npts<32 else -inf.  The device
computes partial maxes of S_p[o] = max_n (r_n . A)[o]; the cheap
elementwise epilogue (plane fold, half fold, -Q, max C, relu,
unpermute) runs on host.

Device structure: pillars sorted by npts desc, 10 slots x 512 pillars
per core; slot i runs B=ceil(maxN_i/2) point-pair matmuls (partition =
2x64 channels, free = 512 pillars) into 2-bank PSUM tiles (ring of 4 =
all 8 banks).  PSUM tiles drain in OFFSET PAIRS: Act copy-casts pair
p's tile A to fp16 SBUF (one 1024-col op), and DVE folds that cast
with pair p+1's tile B in one mixed-dtype tensor_max -> 2 fp16 planes;
the one-pair offset means the DVE op's cast input is long since ready,
so the only live dependency is its own psum tile.  4 banks retire per
(1112ns Act + 1192ns DVE) running concurrently; PE, Act and DVE are
all ~balanced (~27us each per core).  Chain seeds / odd tails drain
via DVE copy/reduce or Act cast chosen by a static load balancer.
Planes collect in a per-slot out tile -> HBM; host max-folds the few
planes per pillar.  All T inputs prefetch at kernel start; the
framework's unused const-tile memsets are suppressed (they gate the
init barrier on the slow-booting GPSIMD).  Empirical constraints:
TensorTensor reads at most one PSUM operand; GPSIMD can't touch PSUM
or run TensorTensor; no cross-partition ops (lane-locked engines);
strided Act writes are 5x slow; fp16 TT gets the 2x DVE mode, reduce
does not; matmul out <= 512 free columns (one PSUM bank).
"""

import sys

import numpy as np

sys.path.insert(0, "/opt/trn_rl_repo")

VX, VY = 0.16, 0.16
X_OFF = VX / 2 + 0.0
Y_OFF = VY / 2 + (-39.68)
BN_EPS = 1e-3

P_FULL = 40000
N_PTS = 32
C_OUT = 64
N_CORES = 8
N_SLOTS = 10
TILE_P = 512
P_PAD = N_CORES * N_SLOTS * TILE_P  # 40960

_CACHE = {}


def _plan_slots(sched):
    """Plane layout per slot + tail drain choice.  Returns per-slot
    (n_planes, n_pairs2, tail) with tail in {None,'sv','sa','r2v','r2a',
    'r3av','r3va'}."""
    est_v, est_a = 0.0, 0.0
    plan = []
    for maxN in sched:
        B = (maxN + 1) // 2
        n_pairs2 = B // 4
        rem = B - 4 * n_pairs2
        # offset-1 chain: casts on Act, folds/seed on DVE; the last cast
        # and first B tile ship as unfolded planes
        seed = "v"
        est_a += n_pairs2 * 1112.0
        est_v += n_pairs2 * 1192.0
        n_planes = 2 * n_pairs2 + (2 if n_pairs2 else 0)
        tail = None
        if rem == 1:
            if est_v + 690.0 <= est_a + 570.0:
                tail, dv, da, pl = "sv", 690.0, 0.0, 1
            else:
                tail, dv, da, pl = "sa", 0.0, 570.0, 1
        elif rem == 2:
            if est_v + 1223.0 <= est_a + 1112.0:
                tail, dv, da, pl = "r2v", 1223.0, 0.0, 1
            else:
                tail, dv, da, pl = "r2a", 0.0, 1112.0, 2
        elif rem == 3:
            if max(est_v + 1223.0, est_a + 570.0) <= max(
                est_v + 690.0, est_a + 1112.0
            ):
                tail, dv, da, pl = "r3va", 1223.0, 570.0, 2
            else:
                tail, dv, da, pl = "r3av", 690.0, 1112.0, 3
        if tail is not None:
            est_v += dv
            est_a += da
            n_planes += pl
        plan.append((n_planes, n_pairs2, tail, seed))
    _plan_slots.est = (est_v, est_a)
    return plan


def _build_nc(sched):
    from contextlib import ExitStack

    from concourse import bass, tile
    from concourse import mybir

    f32 = mybir.dt.float32
    f16 = mybir.dt.float16
    # Skip the framework's const-tile memsets (unused by this kernel:
    # activation Copy with float bias reads no const APs).  They run on
    # the slow-booting GPSIMD engine and gate the init barrier ~1.5us.
    _orig_memset = bass.BassGpSimd.memset
    bass.BassGpSimd.memset = lambda self, ap, constant: None
    try:
        nc = bass.Bass()
    finally:
        bass.BassGpSimd.memset = _orig_memset

    plan = _plan_slots(sched)

    T_ds = []
    for i, maxN in enumerate(sched):
        G = (maxN + 7) // 8
        T_ds.append(
            nc.dram_tensor(f"T{i}", [32 * G, TILE_P], f16, kind="ExternalInput")
        )
    S_d = nc.dram_tensor("S", [128, 4, 128], f16, kind="ExternalInput")
    O_ds = [
        nc.dram_tensor(f"O{i}", [128, pl[0], TILE_P], f16, kind="ExternalOutput")
        for i, pl in enumerate(plan)
    ]

    with tile.TileContext(nc) as tc, ExitStack() as ctx:
        stat = ctx.enter_context(tc.tile_pool(name="stat", bufs=1))
        upool = ctx.enter_context(tc.tile_pool(name="upool", bufs=6))
        opool = ctx.enter_context(tc.tile_pool(name="opool", bufs=3))
        psum = ctx.enter_context(
            tc.tile_pool(name="ps", bufs=4, space=bass.MemorySpace.PSUM)
        )

        # prefetch stationaries + ALL slot inputs up front
        s_sb = stat.tile([128, 4, 128], f16)
        nc.sync.dma_start(s_sb[:], S_d[:])


        t_sbs = []
        for i, maxN in enumerate(sched):
            G = (maxN + 7) // 8
            t_sb = stat.tile([32 * G, TILE_P], f16, name=f"t{i}")
            nc.sync.dma_start(t_sb[:], T_ds[i][:])
            t_sbs.append(t_sb)

        for i, maxN in enumerate(sched):
            G = (maxN + 7) // 8
            n_planes, n_pairs2, tail, seed = plan[i]
            t_sb = t_sbs[i]

            pairs = [
                (w, g) for w in range(4) for g in range(G) if 8 * g + 2 * w < maxN
            ]
            B = (maxN + 1) // 2
            assert len(pairs) == B, (i, maxN, pairs)

            def mm2(pt, bank, j):
                w, g = pairs[j]
                nc.tensor.matmul(
                    pt[:, bank, :],
                    s_sb[32 * g : 32 * g + 32, w, :],
                    t_sb[32 * g : 32 * g + 32, :],
                    start=True,
                    stop=True,
                    tile_position=(32 * g, 0),
                )

            out_sb = opool.tile([128, n_planes, TILE_P], f16, name="o")
            idx = 0
            j = 0
            # offset-1 pairs: DVE folds pair p's B tile with the cast of
            # pair p-1's A tile (one pair-cadence of slack); the first B
            # tile is a plain copy, the last cast goes straight to the
            # out tile.
            prev_u = None
            for p in range(n_pairs2):
                pa = psum.tile([128, 2, TILE_P], f32, name="pt")
                mm2(pa, 0, j); mm2(pa, 1, j + 1)
                pb = psum.tile([128, 2, TILE_P], f32, name="pt")
                mm2(pb, 0, j + 2); mm2(pb, 1, j + 3)
                j += 4
                if p == n_pairs2 - 1:
                    nc.scalar.copy(out_sb[:, idx : idx + 2, :], pa[:])
                    idx += 2
                else:
                    u = upool.tile([128, 2, TILE_P], f16, name="u")
                    nc.scalar.copy(u[:], pa[:])
                if prev_u is None:
                    nc.vector.tensor_copy(out_sb[:, idx : idx + 2, :], pb[:])
                else:
                    nc.vector.tensor_max(
                        out_sb[:, idx : idx + 2, :], prev_u[:], pb[:]
                    )
                idx += 2
                if p != n_pairs2 - 1:
                    prev_u = u
            if tail in ("sv", "sa"):
                pt = psum.tile([128, 2, TILE_P], f32, name="pt")
                mm2(pt, 0, j)
                j += 1
                if tail == "sv":
                    nc.vector.tensor_copy(out_sb[:, idx, :], pt[:, 0, :])
                else:
                    nc.scalar.copy(out_sb[:, idx, :], pt[:, 0, :])
                idx += 1
            elif tail in ("r2v", "r2a"):
                pt = psum.tile([128, 2, TILE_P], f32, name="pt")
                mm2(pt, 0, j); mm2(pt, 1, j + 1)
                j += 2
                if tail == "r2v":
                    nc.vector.tensor_reduce(
                        out_sb[:, idx, :],
                        pt[:].transpose([0, 2, 1]),
                        axis=mybir.AxisListType.X,
                        op=mybir.AluOpType.max,
                    )
                    idx += 1
                else:
                    nc.scalar.copy(out_sb[:, idx : idx + 2, :], pt[:])
                    idx += 2
            elif tail in ("r3av", "r3va"):
                pa = psum.tile([128, 2, TILE_P], f32, name="pt")
                mm2(pa, 0, j); mm2(pa, 1, j + 1)
                pb = psum.tile([128, 2, TILE_P], f32, name="pt")
                mm2(pb, 0, j + 2)
                j += 3
                if tail == "r3va":
                    nc.vector.tensor_reduce(
                        out_sb[:, idx, :],
                        pa[:].transpose([0, 2, 1]),
                        axis=mybir.AxisListType.X,
                        op=mybir.AluOpType.max,
                    )
                    nc.scalar.copy(out_sb[:, idx + 1, :], pb[:, 0, :])
                else:
                    nc.scalar.copy(out_sb[:, idx : idx + 2, :], pa[:])
                    nc.vector.tensor_copy(out_sb[:, idx + 2, :], pb[:, 0, :])
                idx += 2 if tail == "r3va" else 3
            assert idx == n_planes and j == B, (i, idx, n_planes, j, B)
            nc.sync.dma_start(O_ds[i][:], out_sb[:])

    nc.finalize()
    import bass_rust

    # walrus codegen allows at most 1 sync wait per instruction
    bass_rust.generate_event_semaphores(nc)
    return nc


def _plan(voxels, W, gamma, beta, running_mean, running_var,
          voxel_num_points, voxel_coords):
    V = voxels.astype(np.float64)
    npts = voxel_num_points.astype(np.int64)
    coords = voxel_coords.astype(np.float64)
    W64 = W.astype(np.float64)
    s = gamma.astype(np.float64) / np.sqrt(running_var.astype(np.float64) + BN_EPS)
    c0 = beta.astype(np.float64) - running_mean.astype(np.float64) * s

    A = np.stack([
        s * (W64[:, 0] + W64[:, 4] + W64[:, 7]),
        s * (W64[:, 1] + W64[:, 5] + W64[:, 8]),
        s * (W64[:, 2] + W64[:, 6]),
        s * W64[:, 3],
    ], axis=0)  # [4,64]

    cx = coords[:, 3] * VX + X_OFF
    cy = coords[:, 2] * VY + Y_OFF
    m = V[:, :, :3].sum(axis=1) / npts[:, None]
    q = (cx[:, None] * (s * (W64[:, 0] + W64[:, 7]))[None, :]
         + cy[:, None] * (s * (W64[:, 1] + W64[:, 8]))[None, :]
         + m[:, 0:1] * (s * W64[:, 4])[None, :]
         + m[:, 1:2] * (s * W64[:, 5])[None, :]
         + m[:, 2:3] * (s * W64[:, 6])[None, :])
    Q = (q - c0[None, :]).astype(np.float32)                    # [P,64]
    C = np.where((npts < N_PTS)[:, None], c0[None, :], -1e30).astype(np.float32)

    Vmod = voxels.astype(np.float16).copy()
    invalid = np.arange(N_PTS)[None, :] >= npts[:, None]
    Vmod[invalid] = np.broadcast_to(Vmod[:, 0:1, :], Vmod.shape)[invalid]

    pad = P_PAD - P_FULL
    Vp = np.concatenate([Vmod, np.zeros((pad, N_PTS, 4), np.float16)], axis=0)
    Qp = np.concatenate([Q, np.zeros((pad, C_OUT), np.float32)], axis=0)
    Cp = np.concatenate([C, np.zeros((pad, C_OUT), np.float32)], axis=0)
    np_pad = np.concatenate([npts, np.ones(pad, np.int64)])

    order = np.argsort(-np_pad, kind="stable")
    ns = np_pad[order]
    sched = tuple(int(ns[N_CORES * TILE_P * i]) for i in range(N_SLOTS))

    # stationaries: S[32g+4j+c, w, m] = A[c, m%64] if j == 2w + m//64
    A16 = A.astype(np.float16)
    S_small = np.zeros((32, 4, 128), np.float16)
    for w in range(4):
        for half in range(2):
            jj = 2 * w + half
            S_small[4 * jj : 4 * jj + 4, w, 64 * half : 64 * half + 64] = A16
    S = np.tile(S_small, (4, 1, 1))  # [128,4,128]

    Vs = Vp[order]
    in_maps = []
    for k in range(N_CORES):
        mp = {"S": S}
        for i, maxN in enumerate(sched):
            G = (maxN + 7) // 8
            c = N_CORES * i + k
            sl = slice(TILE_P * c, TILE_P * (c + 1))
            mp[f"T{i}"] = np.ascontiguousarray(
                Vs[sl][:, : 8 * G, :].transpose(1, 2, 0).reshape(32 * G, TILE_P)
            )
        in_maps.append(mp)
    return in_maps, sched, order, Qp[order], Cp[order]


def _gather(results, sched, order, Qs, Cs):
    smax = np.empty((P_PAD, C_OUT), np.float32)
    for k in range(N_CORES):
        for i in range(N_SLOTS):
            Ok = results[k][f"O{i}"]  # [128, n_planes, 512] fp16
            pm = Ok.max(axis=1)       # [128, 512]
            fold = np.maximum(pm[:C_OUT, :], pm[C_OUT:, :]).astype(np.float32)
            c = N_CORES * i + k
            smax[TILE_P * c : TILE_P * (c + 1)] = fold.T
    out_sorted = np.maximum(np.maximum(smax - Qs, Cs), 0.0)
    out_full = np.empty_like(out_sorted)
    out_full[order] = out_sorted
    return np.ascontiguousarray(out_full[:P_FULL])


def kernel(**inputs):
    from concourse.bass_utils import run_bass_kernel_spmd

    in_maps, sched, order, Qs, Cs = _plan(**inputs)
    if sched not in _CACHE:
        _CACHE[sched] = _build_nc(sched)
    res = run_bass_kernel_spmd(_CACHE[sched], in_maps, list(range(N_CORES)))
    return _gather(res.results, sched, order, Qs, Cs)


# revision 43
# speedup vs baseline: 1.1790x; 1.0118x over previous
"""PillarVFE on 8 trn2 NeuronCores — v6: fp16 matmuls + Act/DVE chain-pair
PSUM drain, plane outputs, epilogue on host.

Math: per pillar p, point n with raw r=(x,y,z,w):
  out[p,o] = relu( max( max_n (r_n . A)[o] - Q_p[o],  C_p[o] ) )
where A[4,64] folds W + BN scale, Q_p folds the pillar-constant part
(center offsets + cluster mean) minus the BN bias, and C_p is the
candidate from masked points: c0 if npts<32 else -inf.  The device
computes partial maxes of S_p[o] = max_n (r_n . A)[o]; the cheap
elementwise epilogue (plane fold, half fold, -Q, max C, relu,
unpermute) runs on host.

Device structure: pillars sorted by npts desc, 10 slots x 512 pillars
per core; slot i runs B=ceil(maxN_i/2) point-pair matmuls (partition =
2x64 channels, free = 512 pillars) into 2-bank PSUM tiles (ring of 4 =
all 8 banks).  PSUM tiles drain in OFFSET PAIRS: Act copy-casts pair
p's tile A to fp16 SBUF (one 1024-col op), and DVE folds that cast
with pair p+1's tile B in one mixed-dtype tensor_max -> 2 fp16 planes;
the one-pair offset means the DVE op's cast input is long since ready,
so the only live dependency is its own psum tile.  4 banks retire per
(1112ns Act + 1192ns DVE) running concurrently; PE, Act and DVE are
all ~balanced (~27us each per core).  Chain seeds / odd tails drain
via DVE copy/reduce or Act cast chosen by a static load balancer.
Planes collect in a per-slot out tile -> HBM; host max-folds the few
planes per pillar.  All T inputs prefetch at kernel start; the
framework's unused const-tile memsets are suppressed (they gate the
init barrier on the slow-booting GPSIMD).  Empirical constraints:
TensorTensor reads at most one PSUM operand; GPSIMD can't touch PSUM
or run TensorTensor; no cross-partition ops (lane-locked engines);
strided Act writes are 5x slow; fp16 TT gets the 2x DVE mode, reduce
does not; matmul out <= 512 free columns (one PSUM bank).
"""

import sys

import numpy as np

sys.path.insert(0, "/opt/trn_rl_repo")

VX, VY = 0.16, 0.16
X_OFF = VX / 2 + 0.0
Y_OFF = VY / 2 + (-39.68)
BN_EPS = 1e-3

P_FULL = 40000
N_PTS = 32
C_OUT = 64
N_CORES = 8
N_SLOTS = 10
TILE_P = 512
P_PAD = N_CORES * N_SLOTS * TILE_P  # 40960

_CACHE = {}


def _plan_slots(sched):
    """Plane layout per slot + tail drain choice.  Returns per-slot
    (n_planes, n_pairs2, tail) with tail in {None,'sv','sa','r2v','r2a',
    'r3av','r3va'}."""
    est_v, est_a = 0.0, 0.0
    plan = []
    for maxN in sched:
        B = (maxN + 1) // 2
        n_pairs2 = B // 4
        rem = B - 4 * n_pairs2
        # offset-1 chain: casts on Act, folds/seed on DVE; the last cast
        # and first B tile ship as unfolded planes
        seed = "v"
        est_a += n_pairs2 * 1112.0
        est_v += n_pairs2 * 1192.0
        n_planes = 2 * n_pairs2 + (2 if n_pairs2 else 0)
        tail = None
        if rem == 1:
            if est_v + 690.0 <= est_a + 570.0:
                tail, dv, da, pl = "sv", 690.0, 0.0, 1
            else:
                tail, dv, da, pl = "sa", 0.0, 570.0, 1
        elif rem == 2:
            if est_v + 1223.0 <= est_a + 1112.0:
                tail, dv, da, pl = "r2v", 1223.0, 0.0, 1
            else:
                tail, dv, da, pl = "r2a", 0.0, 1112.0, 2
        elif rem == 3:
            if max(est_v + 1223.0, est_a + 570.0) <= max(
                est_v + 690.0, est_a + 1112.0
            ):
                tail, dv, da, pl = "r3va", 1223.0, 570.0, 2
            else:
                tail, dv, da, pl = "r3av", 690.0, 1112.0, 3
        if tail is not None:
            est_v += dv
            est_a += da
            n_planes += pl
        plan.append((n_planes, n_pairs2, tail, seed))
    _plan_slots.est = (est_v, est_a)
    return plan


def _build_nc(sched):
    from contextlib import ExitStack

    from concourse import bass, tile
    from concourse import mybir

    f32 = mybir.dt.float32
    f16 = mybir.dt.float16
    # Skip the framework's const-tile memsets (unused by this kernel:
    # activation Copy with float bias reads no const APs).  They run on
    # the slow-booting GPSIMD engine and gate the init barrier ~1.5us.
    _orig_memset = bass.BassGpSimd.memset
    bass.BassGpSimd.memset = lambda self, ap, constant: None
    try:
        nc = bass.Bass()
    finally:
        bass.BassGpSimd.memset = _orig_memset

    plan = _plan_slots(sched)

    T_ds = []
    for i, maxN in enumerate(sched):
        G = (maxN + 7) // 8
        T_ds.append(
            nc.dram_tensor(f"T{i}", [32 * G, TILE_P], f16, kind="ExternalInput")
        )
    S_d = nc.dram_tensor("S", [128, 4, 128], f16, kind="ExternalInput")
    O_ds = [
        nc.dram_tensor(f"O{i}", [128, pl[0], TILE_P], f16, kind="ExternalOutput")
        for i, pl in enumerate(plan)
    ]

    with tile.TileContext(nc) as tc, ExitStack() as ctx:
        stat = ctx.enter_context(tc.tile_pool(name="stat", bufs=1))
        upool = ctx.enter_context(tc.tile_pool(name="upool", bufs=6))
        opool = ctx.enter_context(tc.tile_pool(name="opool", bufs=3))
        psum = ctx.enter_context(
            tc.tile_pool(name="ps", bufs=4, space=bass.MemorySpace.PSUM)
        )

        # prefetch stationaries + ALL slot inputs up front
        s_sb = stat.tile([128, 4, 128], f16)
        nc.sync.dma_start(s_sb[:], S_d[:])


        t_sbs = []
        for i, maxN in enumerate(sched):
            G = (maxN + 7) // 8
            t_sb = stat.tile([32 * G, TILE_P], f16, name=f"t{i}")
            # slot 0's input goes via the (startup-idle) Act engine's
            # HWDGE queue, in parallel with S on sync
            eng = nc.scalar if i == 0 else nc.sync
            eng.dma_start(t_sb[:], T_ds[i][:])
            t_sbs.append(t_sb)

        for i, maxN in enumerate(sched):
            G = (maxN + 7) // 8
            n_planes, n_pairs2, tail, seed = plan[i]
            t_sb = t_sbs[i]

            pairs = [
                (w, g) for w in range(4) for g in range(G) if 8 * g + 2 * w < maxN
            ]
            B = (maxN + 1) // 2
            assert len(pairs) == B, (i, maxN, pairs)

            def mm2(pt, bank, j):
                w, g = pairs[j]
                nc.tensor.matmul(
                    pt[:, bank, :],
                    s_sb[32 * g : 32 * g + 32, w, :],
                    t_sb[32 * g : 32 * g + 32, :],
                    start=True,
                    stop=True,
                    tile_position=(32 * g, 0),
                )

            out_sb = opool.tile([128, n_planes, TILE_P], f16, name="o")
            idx = 0
            j = 0
            # offset-1 pairs: DVE folds pair p's B tile with the cast of
            # pair p-1's A tile (one pair-cadence of slack); the first B
            # tile is a plain copy, the last cast goes straight to the
            # out tile.
            prev_u = None
            for p in range(n_pairs2):
                pa = psum.tile([128, 2, TILE_P], f32, name="pt")
                mm2(pa, 0, j); mm2(pa, 1, j + 1)
                pb = psum.tile([128, 2, TILE_P], f32, name="pt")
                mm2(pb, 0, j + 2); mm2(pb, 1, j + 3)
                j += 4
                if p == n_pairs2 - 1:
                    nc.scalar.copy(out_sb[:, idx : idx + 2, :], pa[:])
                    idx += 2
                else:
                    u = upool.tile([128, 2, TILE_P], f16, name="u")
                    nc.scalar.copy(u[:], pa[:])
                if prev_u is None:
                    nc.vector.tensor_copy(out_sb[:, idx : idx + 2, :], pb[:])
                else:
                    nc.vector.tensor_max(
                        out_sb[:, idx : idx + 2, :], prev_u[:], pb[:]
                    )
                idx += 2
                if p != n_pairs2 - 1:
                    prev_u = u
            if tail in ("sv", "sa"):
                pt = psum.tile([128, 2, TILE_P], f32, name="pt")
                mm2(pt, 0, j)
                j += 1
                if tail == "sv":
                    nc.vector.tensor_copy(out_sb[:, idx, :], pt[:, 0, :])
                else:
                    nc.scalar.copy(out_sb[:, idx, :], pt[:, 0, :])
                idx += 1
            elif tail in ("r2v", "r2a"):
                pt = psum.tile([128, 2, TILE_P], f32, name="pt")
                mm2(pt, 0, j); mm2(pt, 1, j + 1)
                j += 2
                if tail == "r2v":
                    nc.vector.tensor_reduce(
                        out_sb[:, idx, :],
                        pt[:].transpose([0, 2, 1]),
                        axis=mybir.AxisListType.X,
                        op=mybir.AluOpType.max,
                    )
                    idx += 1
                else:
                    nc.scalar.copy(out_sb[:, idx : idx + 2, :], pt[:])
                    idx += 2
            elif tail in ("r3av", "r3va"):
                pa = psum.tile([128, 2, TILE_P], f32, name="pt")
                mm2(pa, 0, j); mm2(pa, 1, j + 1)
                pb = psum.tile([128, 2, TILE_P], f32, name="pt")
                mm2(pb, 0, j + 2)
                j += 3
                if tail == "r3va":
                    nc.vector.tensor_reduce(
                        out_sb[:, idx, :],
                        pa[:].transpose([0, 2, 1]),
                        axis=mybir.AxisListType.X,
                        op=mybir.AluOpType.max,
                    )
                    nc.scalar.copy(out_sb[:, idx + 1, :], pb[:, 0, :])
                else:
                    nc.scalar.copy(out_sb[:, idx : idx + 2, :], pa[:])
                    nc.vector.tensor_copy(out_sb[:, idx + 2, :], pb[:, 0, :])
                idx += 2 if tail == "r3va" else 3
            assert idx == n_planes and j == B, (i, idx, n_planes, j, B)
            nc.sync.dma_start(O_ds[i][:], out_sb[:])

    nc.finalize()
    import bass_rust

    # walrus codegen allows at most 1 sync wait per instruction
    bass_rust.generate_event_semaphores(nc)
    return nc


def _plan(voxels, W, gamma, beta, running_mean, running_var,
          voxel_num_points, voxel_coords):
    V = voxels.astype(np.float64)
    npts = voxel_num_points.astype(np.int64)
    coords = voxel_coords.astype(np.float64)
    W64 = W.astype(np.float64)
    s = gamma.astype(np.float64) / np.sqrt(running_var.astype(np.float64) + BN_EPS)
    c0 = beta.astype(np.float64) - running_mean.astype(np.float64) * s

    A = np.stack([
        s * (W64[:, 0] + W64[:, 4] + W64[:, 7]),
        s * (W64[:, 1] + W64[:, 5] + W64[:, 8]),
        s * (W64[:, 2] + W64[:, 6]),
        s * W64[:, 3],
    ], axis=0)  # [4,64]

    cx = coords[:, 3] * VX + X_OFF
    cy = coords[:, 2] * VY + Y_OFF
    m = V[:, :, :3].sum(axis=1) / npts[:, None]
    q = (cx[:, None] * (s * (W64[:, 0] + W64[:, 7]))[None, :]
         + cy[:, None] * (s * (W64[:, 1] + W64[:, 8]))[None, :]
         + m[:, 0:1] * (s * W64[:, 4])[None, :]
         + m[:, 1:2] * (s * W64[:, 5])[None, :]
         + m[:, 2:3] * (s * W64[:, 6])[None, :])
    Q = (q - c0[None, :]).astype(np.float32)                    # [P,64]
    C = np.where((npts < N_PTS)[:, None], c0[None, :], -1e30).astype(np.float32)

    Vmod = voxels.astype(np.float16).copy()
    invalid = np.arange(N_PTS)[None, :] >= npts[:, None]
    Vmod[invalid] = np.broadcast_to(Vmod[:, 0:1, :], Vmod.shape)[invalid]

    pad = P_PAD - P_FULL
    Vp = np.concatenate([Vmod, np.zeros((pad, N_PTS, 4), np.float16)], axis=0)
    Qp = np.concatenate([Q, np.zeros((pad, C_OUT), np.float32)], axis=0)
    Cp = np.concatenate([C, np.zeros((pad, C_OUT), np.float32)], axis=0)
    np_pad = np.concatenate([npts, np.ones(pad, np.int64)])

    order = np.argsort(-np_pad, kind="stable")
    ns = np_pad[order]
    sched = tuple(int(ns[N_CORES * TILE_P * i]) for i in range(N_SLOTS))

    # stationaries: S[32g+4j+c, w, m] = A[c, m%64] if j == 2w + m//64
    A16 = A.astype(np.float16)
    S_small = np.zeros((32, 4, 128), np.float16)
    for w in range(4):
        for half in range(2):
            jj = 2 * w + half
            S_small[4 * jj : 4 * jj + 4, w, 64 * half : 64 * half + 64] = A16
    S = np.tile(S_small, (4, 1, 1))  # [128,4,128]

    Vs = Vp[order]
    in_maps = []
    for k in range(N_CORES):
        mp = {"S": S}
        for i, maxN in enumerate(sched):
            G = (maxN + 7) // 8
            c = N_CORES * i + k
            sl = slice(TILE_P * c, TILE_P * (c + 1))
            mp[f"T{i}"] = np.ascontiguousarray(
                Vs[sl][:, : 8 * G, :].transpose(1, 2, 0).reshape(32 * G, TILE_P)
            )
        in_maps.append(mp)
    return in_maps, sched, order, Qp[order], Cp[order]


def _gather(results, sched, order, Qs, Cs):
    smax = np.empty((P_PAD, C_OUT), np.float32)
    for k in range(N_CORES):
        for i in range(N_SLOTS):
            Ok = results[k][f"O{i}"]  # [128, n_planes, 512] fp16
            pm = Ok.max(axis=1)       # [128, 512]
            fold = np.maximum(pm[:C_OUT, :], pm[C_OUT:, :]).astype(np.float32)
            c = N_CORES * i + k
            smax[TILE_P * c : TILE_P * (c + 1)] = fold.T
    out_sorted = np.maximum(np.maximum(smax - Qs, Cs), 0.0)
    out_full = np.empty_like(out_sorted)
    out_full[order] = out_sorted
    return np.ascontiguousarray(out_full[:P_FULL])


def kernel(**inputs):
    from concourse.bass_utils import run_bass_kernel_spmd

    in_maps, sched, order, Qs, Cs = _plan(**inputs)
    if sched not in _CACHE:
        _CACHE[sched] = _build_nc(sched)
    res = run_bass_kernel_spmd(_CACHE[sched], in_maps, list(range(N_CORES)))
    return _gather(res.results, sched, order, Qs, Cs)


# revision 44
# speedup vs baseline: 1.2172x; 1.0324x over previous
"""PillarVFE on 8 trn2 NeuronCores — v6: fp16 matmuls + Act/DVE chain-pair
PSUM drain, plane outputs, epilogue on host.

Math: per pillar p, point n with raw r=(x,y,z,w):
  out[p,o] = relu( max( max_n (r_n . A)[o] - Q_p[o],  C_p[o] ) )
where A[4,64] folds W + BN scale, Q_p folds the pillar-constant part
(center offsets + cluster mean) minus the BN bias, and C_p is the
candidate from masked points: c0 if npts<32 else -inf.  The device
computes partial maxes of S_p[o] = max_n (r_n . A)[o]; the cheap
elementwise epilogue (plane fold, half fold, -Q, max C, relu,
unpermute) runs on host.

Device structure: pillars sorted by npts desc, 10 slots x 512 pillars
per core; slot i runs B=ceil(maxN_i/2) point-pair matmuls (partition =
2x64 channels, free = 512 pillars) into 2-bank PSUM tiles (ring of 4 =
all 8 banks).  PSUM tiles drain in OFFSET PAIRS: Act copy-casts pair
p's tile A to fp16 SBUF (one 1024-col op), and DVE folds that cast
with pair p+1's tile B in one mixed-dtype tensor_max -> 2 fp16 planes;
the one-pair offset means the DVE op's cast input is long since ready,
so the only live dependency is its own psum tile.  4 banks retire per
(1112ns Act + 1192ns DVE) running concurrently; PE, Act and DVE are
all ~balanced (~27us each per core).  Chain seeds / odd tails drain
via DVE copy/reduce or Act cast chosen by a static load balancer.
Planes collect in a per-slot out tile -> HBM; host max-folds the few
planes per pillar.  All T inputs prefetch at kernel start (slot 0's
via the startup-idle Act HWDGE queue, in parallel with S on sync); the
framework's unused const-tile memsets are suppressed (they gate the
init barrier on the slow-booting GPSIMD).  Empirical constraints:
TensorTensor reads at most one PSUM operand; GPSIMD can't touch PSUM
or run TensorTensor; no cross-partition ops (lane-locked engines);
strided Act writes are 5x slow; fp16 TT gets the 2x DVE mode, reduce
does not; matmul out <= 512 free columns (one PSUM bank).
"""

import sys

import numpy as np

sys.path.insert(0, "/opt/trn_rl_repo")

VX, VY = 0.16, 0.16
X_OFF = VX / 2 + 0.0
Y_OFF = VY / 2 + (-39.68)
BN_EPS = 1e-3

P_FULL = 40000
N_PTS = 32
C_OUT = 64
N_CORES = 8
N_SLOTS = 10
TILE_P = 512
P_PAD = N_CORES * N_SLOTS * TILE_P  # 40960

_CACHE = {}


def _plan_slots(sched):
    """Plane layout per slot + tail drain choice.  Returns per-slot
    (n_planes, n_pairs2, tail) with tail in {None,'sv','sa','r2v','r2a',
    'r3av','r3va'}."""
    est_v, est_a = 0.0, 0.0
    plan = []
    for maxN in sched:
        B = (maxN + 1) // 2
        n_pairs2 = B // 4
        rem = B - 4 * n_pairs2
        # offset-1 chain: casts on Act, folds/seed on DVE; the last cast
        # and first B tile ship as unfolded planes
        seed = "v"
        est_a += n_pairs2 * 1112.0
        est_v += n_pairs2 * 1192.0
        n_planes = 2 * n_pairs2 + (2 if n_pairs2 else 0)
        tail = None
        if rem == 1:
            if est_v + 690.0 <= est_a + 570.0:
                tail, dv, da, pl = "sv", 690.0, 0.0, 1
            else:
                tail, dv, da, pl = "sa", 0.0, 570.0, 1
        elif rem == 2:
            if est_v + 1223.0 <= est_a + 1112.0:
                tail, dv, da, pl = "r2v", 1223.0, 0.0, 1
            else:
                tail, dv, da, pl = "r2a", 0.0, 1112.0, 2
        elif rem == 3:
            if max(est_v + 1223.0, est_a + 570.0) <= max(
                est_v + 690.0, est_a + 1112.0
            ):
                tail, dv, da, pl = "r3va", 1223.0, 570.0, 2
            else:
                tail, dv, da, pl = "r3av", 690.0, 1112.0, 3
        if tail is not None:
            est_v += dv
            est_a += da
            n_planes += pl
        plan.append((n_planes, n_pairs2, tail, seed))
    _plan_slots.est = (est_v, est_a)
    return plan


def _build_nc(sched):
    from contextlib import ExitStack

    from concourse import bass, tile
    from concourse import mybir

    f32 = mybir.dt.float32
    f16 = mybir.dt.float16
    # Skip the framework's const-tile memsets (unused by this kernel:
    # activation Copy with float bias reads no const APs).  They run on
    # the slow-booting GPSIMD engine and gate the init barrier ~1.5us.
    _orig_memset = bass.BassGpSimd.memset
    bass.BassGpSimd.memset = lambda self, ap, constant: None
    try:
        nc = bass.Bass()
    finally:
        bass.BassGpSimd.memset = _orig_memset

    plan = _plan_slots(sched)

    T_ds = []
    for i, maxN in enumerate(sched):
        G = (maxN + 7) // 8
        T_ds.append(
            nc.dram_tensor(f"T{i}", [32 * G, TILE_P], f16, kind="ExternalInput")
        )
    S_d = nc.dram_tensor("S", [128, 4, 128], f16, kind="ExternalInput")
    O_ds = [
        nc.dram_tensor(f"O{i}", [128, pl[0], TILE_P], f16, kind="ExternalOutput")
        for i, pl in enumerate(plan)
    ]

    with tile.TileContext(nc) as tc, ExitStack() as ctx:
        stat = ctx.enter_context(tc.tile_pool(name="stat", bufs=1))
        upool = ctx.enter_context(tc.tile_pool(name="upool", bufs=6))
        opool = ctx.enter_context(tc.tile_pool(name="opool", bufs=3))
        psum = ctx.enter_context(
            tc.tile_pool(name="ps", bufs=4, space=bass.MemorySpace.PSUM)
        )

        # prefetch stationaries + ALL slot inputs up front
        s_sb = stat.tile([128, 4, 128], f16)
        nc.sync.dma_start(s_sb[:], S_d[:])


        t_sbs = []
        for i, maxN in enumerate(sched):
            G = (maxN + 7) // 8
            t_sb = stat.tile([32 * G, TILE_P], f16, name=f"t{i}")
            # slot 0's input goes via the (startup-idle) Act engine's
            # HWDGE queue, in parallel with S on sync
            eng = nc.scalar if i == 0 else nc.sync
            eng.dma_start(t_sb[:], T_ds[i][:])
            t_sbs.append(t_sb)

        for i, maxN in enumerate(sched):
            G = (maxN + 7) // 8
            n_planes, n_pairs2, tail, seed = plan[i]
            t_sb = t_sbs[i]

            pairs = [
                (w, g) for w in range(4) for g in range(G) if 8 * g + 2 * w < maxN
            ]
            B = (maxN + 1) // 2
            assert len(pairs) == B, (i, maxN, pairs)

            def mm2(pt, bank, j):
                w, g = pairs[j]
                nc.tensor.matmul(
                    pt[:, bank, :],
                    s_sb[32 * g : 32 * g + 32, w, :],
                    t_sb[32 * g : 32 * g + 32, :],
                    start=True,
                    stop=True,
                    tile_position=(32 * g, 0),
                )

            out_sb = opool.tile([128, n_planes, TILE_P], f16, name="o")
            idx = 0
            j = 0
            # offset-1 pairs: DVE folds pair p's B tile with the cast of
            # pair p-1's A tile (one pair-cadence of slack); the first B
            # tile is a plain copy, the last cast goes straight to the
            # out tile.
            prev_u = None
            for p in range(n_pairs2):
                pa = psum.tile([128, 2, TILE_P], f32, name="pt")
                mm2(pa, 0, j); mm2(pa, 1, j + 1)
                pb = psum.tile([128, 2, TILE_P], f32, name="pt")
                mm2(pb, 0, j + 2); mm2(pb, 1, j + 3)
                j += 4
                if p == n_pairs2 - 1:
                    nc.scalar.copy(out_sb[:, idx : idx + 2, :], pa[:])
                    idx += 2
                else:
                    u = upool.tile([128, 2, TILE_P], f16, name="u")
                    nc.scalar.copy(u[:], pa[:])
                if prev_u is None:
                    nc.vector.tensor_copy(out_sb[:, idx : idx + 2, :], pb[:])
                else:
                    nc.vector.tensor_max(
                        out_sb[:, idx : idx + 2, :], prev_u[:], pb[:]
                    )
                idx += 2
                if p != n_pairs2 - 1:
                    prev_u = u
            if tail in ("sv", "sa"):
                pt = psum.tile([128, 2, TILE_P], f32, name="pt")
                mm2(pt, 0, j)
                j += 1
                if tail == "sv":
                    nc.vector.tensor_copy(out_sb[:, idx, :], pt[:, 0, :])
                else:
                    nc.scalar.copy(out_sb[:, idx, :], pt[:, 0, :])
                idx += 1
            elif tail in ("r2v", "r2a"):
                pt = psum.tile([128, 2, TILE_P], f32, name="pt")
                mm2(pt, 0, j); mm2(pt, 1, j + 1)
                j += 2
                if tail == "r2v":
                    nc.vector.tensor_reduce(
                        out_sb[:, idx, :],
                        pt[:].transpose([0, 2, 1]),
                        axis=mybir.AxisListType.X,
                        op=mybir.AluOpType.max,
                    )
                    idx += 1
                else:
                    nc.scalar.copy(out_sb[:, idx : idx + 2, :], pt[:])
                    idx += 2
            elif tail in ("r3av", "r3va"):
                pa = psum.tile([128, 2, TILE_P], f32, name="pt")
                mm2(pa, 0, j); mm2(pa, 1, j + 1)
                pb = psum.tile([128, 2, TILE_P], f32, name="pt")
                mm2(pb, 0, j + 2)
                j += 3
                if tail == "r3va":
                    nc.vector.tensor_reduce(
                        out_sb[:, idx, :],
                        pa[:].transpose([0, 2, 1]),
                        axis=mybir.AxisListType.X,
                        op=mybir.AluOpType.max,
                    )
                    nc.scalar.copy(out_sb[:, idx + 1, :], pb[:, 0, :])
                else:
                    nc.scalar.copy(out_sb[:, idx : idx + 2, :], pa[:])
                    nc.vector.tensor_copy(out_sb[:, idx + 2, :], pb[:, 0, :])
                idx += 2 if tail == "r3va" else 3
            assert idx == n_planes and j == B, (i, idx, n_planes, j, B)
            nc.sync.dma_start(O_ds[i][:], out_sb[:])

    nc.finalize()
    import bass_rust

    # walrus codegen allows at most 1 sync wait per instruction
    bass_rust.generate_event_semaphores(nc)
    return nc


def _plan(voxels, W, gamma, beta, running_mean, running_var,
          voxel_num_points, voxel_coords):
    V = voxels.astype(np.float64)
    npts = voxel_num_points.astype(np.int64)
    coords = voxel_coords.astype(np.float64)
    W64 = W.astype(np.float64)
    s = gamma.astype(np.float64) / np.sqrt(running_var.astype(np.float64) + BN_EPS)
    c0 = beta.astype(np.float64) - running_mean.astype(np.float64) * s

    A = np.stack([
        s * (W64[:, 0] + W64[:, 4] + W64[:, 7]),
        s * (W64[:, 1] + W64[:, 5] + W64[:, 8]),
        s * (W64[:, 2] + W64[:, 6]),
        s * W64[:, 3],
    ], axis=0)  # [4,64]

    cx = coords[:, 3] * VX + X_OFF
    cy = coords[:, 2] * VY + Y_OFF
    m = V[:, :, :3].sum(axis=1) / npts[:, None]
    q = (cx[:, None] * (s * (W64[:, 0] + W64[:, 7]))[None, :]
         + cy[:, None] * (s * (W64[:, 1] + W64[:, 8]))[None, :]
         + m[:, 0:1] * (s * W64[:, 4])[None, :]
         + m[:, 1:2] * (s * W64[:, 5])[None, :]
         + m[:, 2:3] * (s * W64[:, 6])[None, :])
    Q = (q - c0[None, :]).astype(np.float32)                    # [P,64]
    C = np.where((npts < N_PTS)[:, None], c0[None, :], -1e30).astype(np.float32)

    Vmod = voxels.astype(np.float16).copy()
    invalid = np.arange(N_PTS)[None, :] >= npts[:, None]
    Vmod[invalid] = np.broadcast_to(Vmod[:, 0:1, :], Vmod.shape)[invalid]

    pad = P_PAD - P_FULL
    Vp = np.concatenate([Vmod, np.zeros((pad, N_PTS, 4), np.float16)], axis=0)
    Qp = np.concatenate([Q, np.zeros((pad, C_OUT), np.float32)], axis=0)
    Cp = np.concatenate([C, np.zeros((pad, C_OUT), np.float32)], axis=0)
    np_pad = np.concatenate([npts, np.ones(pad, np.int64)])

    order = np.argsort(-np_pad, kind="stable")
    ns = np_pad[order]
    sched = tuple(int(ns[N_CORES * TILE_P * i]) for i in range(N_SLOTS))

    # stationaries: S[32g+4j+c, w, m] = A[c, m%64] if j == 2w + m//64
    A16 = A.astype(np.float16)
    S_small = np.zeros((32, 4, 128), np.float16)
    for w in range(4):
        for half in range(2):
            jj = 2 * w + half
            S_small[4 * jj : 4 * jj + 4, w, 64 * half : 64 * half + 64] = A16
    S = np.tile(S_small, (4, 1, 1))  # [128,4,128]

    Vs = Vp[order]
    in_maps = []
    for k in range(N_CORES):
        mp = {"S": S}
        for i, maxN in enumerate(sched):
            G = (maxN + 7) // 8
            c = N_CORES * i + k
            sl = slice(TILE_P * c, TILE_P * (c + 1))
            mp[f"T{i}"] = np.ascontiguousarray(
                Vs[sl][:, : 8 * G, :].transpose(1, 2, 0).reshape(32 * G, TILE_P)
            )
        in_maps.append(mp)
    return in_maps, sched, order, Qp[order], Cp[order]


def _gather(results, sched, order, Qs, Cs):
    smax = np.empty((P_PAD, C_OUT), np.float32)
    for k in range(N_CORES):
        for i in range(N_SLOTS):
            Ok = results[k][f"O{i}"]  # [128, n_planes, 512] fp16
            pm = Ok.max(axis=1)       # [128, 512]
            fold = np.maximum(pm[:C_OUT, :], pm[C_OUT:, :]).astype(np.float32)
            c = N_CORES * i + k
            smax[TILE_P * c : TILE_P * (c + 1)] = fold.T
    out_sorted = np.maximum(np.maximum(smax - Qs, Cs), 0.0)
    out_full = np.empty_like(out_sorted)
    out_full[order] = out_sorted
    return np.ascontiguousarray(out_full[:P_FULL])


def kernel(**inputs):
    from concourse.bass_utils import run_bass_kernel_spmd

    in_maps, sched, order, Qs, Cs = _plan(**inputs)
    if sched not in _CACHE:
        _CACHE[sched] = _build_nc(sched)
    res = run_bass_kernel_spmd(_CACHE[sched], in_maps, list(range(N_CORES)))
    return _gather(res.results, sched, order, Qs, Cs)


# revision 45
# speedup vs baseline: 1.2229x; 1.0047x over previous
"""PillarVFE on 8 trn2 NeuronCores — v6: fp16 matmuls + Act/DVE chain-pair
PSUM drain, plane outputs, epilogue on host.

Math: per pillar p, point n with raw r=(x,y,z,w):
  out[p,o] = relu( max( max_n (r_n . A)[o] - Q_p[o],  C_p[o] ) )
where A[4,64] folds W + BN scale, Q_p folds the pillar-constant part
(center offsets + cluster mean) minus the BN bias, and C_p is the
candidate from masked points: c0 if npts<32 else -inf.  The device
computes partial maxes of S_p[o] = max_n (r_n . A)[o]; the cheap
elementwise epilogue (plane fold, half fold, -Q, max C, relu,
unpermute) runs on host.

Device structure: pillars sorted by npts desc, 10 slots x 512 pillars
per core; slot i runs B=ceil(maxN_i/2) point-pair matmuls (partition =
2x64 channels, free = 512 pillars) into 2-bank PSUM tiles (ring of 4 =
all 8 banks).  PSUM tiles drain in OFFSET PAIRS: Act copy-casts pair
p's tile A to fp16 SBUF (one 1024-col op), and DVE folds that cast
with pair p+1's tile B in one mixed-dtype tensor_max -> 2 fp16 planes;
the one-pair offset means the DVE op's cast input is long since ready,
so the only live dependency is its own psum tile.  4 banks retire per
(1112ns Act + 1192ns DVE) running concurrently; PE, Act and DVE are
all ~balanced (~27us each per core).  Chain seeds / odd tails drain
via DVE copy/reduce or Act cast chosen by a static load balancer.
Planes collect in a per-slot out tile -> HBM; host max-folds the few
planes per pillar.  All T inputs prefetch at kernel start (slot 0's
via the startup-idle Act HWDGE queue, in parallel with S on sync); the
framework's unused const-tile memsets are suppressed (they gate the
init barrier on the slow-booting GPSIMD).  Empirical constraints:
TensorTensor reads at most one PSUM operand; GPSIMD can't touch PSUM
or run TensorTensor; no cross-partition ops (lane-locked engines);
strided Act writes are 5x slow; fp16 TT gets the 2x DVE mode, reduce
does not; matmul out <= 512 free columns (one PSUM bank).
"""

import sys

import numpy as np

sys.path.insert(0, "/opt/trn_rl_repo")

VX, VY = 0.16, 0.16
X_OFF = VX / 2 + 0.0
Y_OFF = VY / 2 + (-39.68)
BN_EPS = 1e-3

P_FULL = 40000
N_PTS = 32
C_OUT = 64
N_CORES = 8
N_SLOTS = 10
TILE_P = 512
P_PAD = N_CORES * N_SLOTS * TILE_P  # 40960

_CACHE = {}


def _plan_slots(sched):
    """Plane layout per slot + tail drain choice.  Returns per-slot
    (n_planes, n_pairs2, tail) with tail in {None,'sv','sa','r2v','r2a',
    'r3av','r3va'}."""
    est_v, est_a = 0.0, 0.0
    plan = []
    for maxN in sched:
        B = (maxN + 1) // 2
        n_pairs2 = B // 4
        rem = B - 4 * n_pairs2
        # offset-1 chain: casts on Act, folds/seed on DVE; the last cast
        # and first B tile ship as unfolded planes
        seed = "v"
        est_a += n_pairs2 * 1112.0
        est_v += n_pairs2 * 1192.0
        n_planes = 2 * n_pairs2 + (2 if n_pairs2 else 0)
        tail = None
        if rem == 1:
            if est_v + 690.0 <= est_a + 570.0:
                tail, dv, da, pl = "sv", 690.0, 0.0, 1
            else:
                tail, dv, da, pl = "sa", 0.0, 570.0, 1
        elif rem == 2:
            if est_v + 1223.0 <= est_a + 1112.0:
                tail, dv, da, pl = "r2v", 1223.0, 0.0, 1
            else:
                tail, dv, da, pl = "r2a", 0.0, 1112.0, 2
        elif rem == 3:
            if max(est_v + 1223.0, est_a + 570.0) <= max(
                est_v + 690.0, est_a + 1112.0
            ):
                tail, dv, da, pl = "r3va", 1223.0, 570.0, 2
            else:
                tail, dv, da, pl = "r3av", 690.0, 1112.0, 3
        if tail is not None:
            est_v += dv
            est_a += da
            n_planes += pl
        plan.append((n_planes, n_pairs2, tail, seed))
    _plan_slots.est = (est_v, est_a)
    return plan


def _build_nc(sched):
    from contextlib import ExitStack

    from concourse import bass, tile
    from concourse import mybir

    f32 = mybir.dt.float32
    f16 = mybir.dt.float16
    # Skip the framework's const-tile memsets (unused by this kernel:
    # activation Copy with float bias reads no const APs).  They run on
    # the slow-booting GPSIMD engine and gate the init barrier ~1.5us.
    _orig_memset = bass.BassGpSimd.memset
    bass.BassGpSimd.memset = lambda self, ap, constant: None
    try:
        nc = bass.Bass()
    finally:
        bass.BassGpSimd.memset = _orig_memset

    plan = _plan_slots(sched)

    T_ds = []
    for i, maxN in enumerate(sched):
        G = (maxN + 7) // 8
        T_ds.append(
            nc.dram_tensor(f"T{i}", [32 * G, TILE_P], f16, kind="ExternalInput")
        )
    S_d = nc.dram_tensor("S", [128, 4, 128], f16, kind="ExternalInput")
    O_ds = [
        nc.dram_tensor(f"O{i}", [128, pl[0], TILE_P], f16, kind="ExternalOutput")
        for i, pl in enumerate(plan)
    ]

    with tile.TileContext(nc) as tc, ExitStack() as ctx:
        stat = ctx.enter_context(tc.tile_pool(name="stat", bufs=1))
        upool = ctx.enter_context(tc.tile_pool(name="upool", bufs=6))
        opool = ctx.enter_context(tc.tile_pool(name="opool", bufs=3))
        psum = ctx.enter_context(
            tc.tile_pool(name="ps", bufs=4, space=bass.MemorySpace.PSUM)
        )

        # prefetch stationaries + ALL slot inputs up front
        s_sb = stat.tile([128, 4, 128], f16)
        nc.sync.dma_start(s_sb[:], S_d[:])


        t_sbs = []
        for i, maxN in enumerate(sched):
            G = (maxN + 7) // 8
            t_sb = stat.tile([32 * G, TILE_P], f16, name=f"t{i}")
            # slot 0's input goes via the (startup-idle) Act engine's
            # HWDGE queue, in parallel with S on sync
            eng = nc.scalar if i == 0 else nc.sync
            eng.dma_start(t_sb[:], T_ds[i][:])
            t_sbs.append(t_sb)

        for i, maxN in enumerate(sched):
            G = (maxN + 7) // 8
            n_planes, n_pairs2, tail, seed = plan[i]
            t_sb = t_sbs[i]

            pairs = [
                (w, g) for w in range(4) for g in range(G) if 8 * g + 2 * w < maxN
            ]
            B = (maxN + 1) // 2
            assert len(pairs) == B, (i, maxN, pairs)

            def mm2(pt, bank, j):
                w, g = pairs[j]
                nc.tensor.matmul(
                    pt[:, bank, :],
                    s_sb[32 * g : 32 * g + 32, w, :],
                    t_sb[32 * g : 32 * g + 32, :],
                    start=True,
                    stop=True,
                    tile_position=(32 * g, 0),
                )

            out_sb = opool.tile([128, n_planes, TILE_P], f16, name="o")
            idx = 0
            j = 0
            # offset-1 pairs: DVE folds pair p's B tile with the cast of
            # pair p-1's A tile (one pair-cadence of slack); the first B
            # tile is a plain copy, the last cast goes straight to the
            # out tile.
            prev_u = None
            for p in range(n_pairs2):
                pa = psum.tile([128, 2, TILE_P], f32, name="pt")
                mm2(pa, 0, j); mm2(pa, 1, j + 1)
                pb = psum.tile([128, 2, TILE_P], f32, name="pt")
                mm2(pb, 0, j + 2); mm2(pb, 1, j + 3)
                j += 4
                if p == n_pairs2 - 1:
                    nc.scalar.copy(out_sb[:, idx : idx + 2, :], pa[:])
                    idx += 2
                else:
                    u = upool.tile([128, 2, TILE_P], f16, name="u")
                    nc.scalar.copy(u[:], pa[:])
                if prev_u is None:
                    nc.vector.tensor_copy(out_sb[:, idx : idx + 2, :], pb[:])
                else:
                    nc.vector.tensor_max(
                        out_sb[:, idx : idx + 2, :], prev_u[:], pb[:]
                    )
                idx += 2
                if p != n_pairs2 - 1:
                    prev_u = u
            if tail in ("sv", "sa"):
                pt = psum.tile([128, 2, TILE_P], f32, name="pt")
                mm2(pt, 0, j)
                j += 1
                if tail == "sv":
                    nc.vector.tensor_copy(out_sb[:, idx, :], pt[:, 0, :])
                else:
                    nc.scalar.copy(out_sb[:, idx, :], pt[:, 0, :])
                idx += 1
            elif tail in ("r2v", "r2a"):
                pt = psum.tile([128, 2, TILE_P], f32, name="pt")
                mm2(pt, 0, j); mm2(pt, 1, j + 1)
                j += 2
                if tail == "r2v":
                    nc.vector.tensor_reduce(
                        out_sb[:, idx, :],
                        pt[:].transpose([0, 2, 1]),
                        axis=mybir.AxisListType.X,
                        op=mybir.AluOpType.max,
                    )
                    idx += 1
                else:
                    nc.scalar.copy(out_sb[:, idx : idx + 2, :], pt[:])
                    idx += 2
            elif tail in ("r3av", "r3va"):
                pa = psum.tile([128, 2, TILE_P], f32, name="pt")
                mm2(pa, 0, j); mm2(pa, 1, j + 1)
                pb = psum.tile([128, 2, TILE_P], f32, name="pt")
                mm2(pb, 0, j + 2)
                j += 3
                if tail == "r3va":
                    nc.vector.tensor_reduce(
                        out_sb[:, idx, :],
                        pa[:].transpose([0, 2, 1]),
                        axis=mybir.AxisListType.X,
                        op=mybir.AluOpType.max,
                    )
                    nc.scalar.copy(out_sb[:, idx + 1, :], pb[:, 0, :])
                else:
                    nc.scalar.copy(out_sb[:, idx : idx + 2, :], pa[:])
                    nc.vector.tensor_copy(out_sb[:, idx + 2, :], pb[:, 0, :])
                idx += 2 if tail == "r3va" else 3
            assert idx == n_planes and j == B, (i, idx, n_planes, j, B)
            nc.sync.dma_start(O_ds[i][:], out_sb[:])

    nc.finalize()
    import bass_rust

    # move extra matmul waits onto the earlier ldweights so matmuls
    # issue immediately once weights are loaded
    bass_rust.move_matmul_waits_to_ldweights(nc.m)
    # walrus codegen allows at most 1 sync wait per instruction
    bass_rust.generate_event_semaphores(nc)
    return nc


def _plan(voxels, W, gamma, beta, running_mean, running_var,
          voxel_num_points, voxel_coords):
    V = voxels.astype(np.float64)
    npts = voxel_num_points.astype(np.int64)
    coords = voxel_coords.astype(np.float64)
    W64 = W.astype(np.float64)
    s = gamma.astype(np.float64) / np.sqrt(running_var.astype(np.float64) + BN_EPS)
    c0 = beta.astype(np.float64) - running_mean.astype(np.float64) * s

    A = np.stack([
        s * (W64[:, 0] + W64[:, 4] + W64[:, 7]),
        s * (W64[:, 1] + W64[:, 5] + W64[:, 8]),
        s * (W64[:, 2] + W64[:, 6]),
        s * W64[:, 3],
    ], axis=0)  # [4,64]

    cx = coords[:, 3] * VX + X_OFF
    cy = coords[:, 2] * VY + Y_OFF
    m = V[:, :, :3].sum(axis=1) / npts[:, None]
    q = (cx[:, None] * (s * (W64[:, 0] + W64[:, 7]))[None, :]
         + cy[:, None] * (s * (W64[:, 1] + W64[:, 8]))[None, :]
         + m[:, 0:1] * (s * W64[:, 4])[None, :]
         + m[:, 1:2] * (s * W64[:, 5])[None, :]
         + m[:, 2:3] * (s * W64[:, 6])[None, :])
    Q = (q - c0[None, :]).astype(np.float32)                    # [P,64]
    C = np.where((npts < N_PTS)[:, None], c0[None, :], -1e30).astype(np.float32)

    Vmod = voxels.astype(np.float16).copy()
    invalid = np.arange(N_PTS)[None, :] >= npts[:, None]
    Vmod[invalid] = np.broadcast_to(Vmod[:, 0:1, :], Vmod.shape)[invalid]

    pad = P_PAD - P_FULL
    Vp = np.concatenate([Vmod, np.zeros((pad, N_PTS, 4), np.float16)], axis=0)
    Qp = np.concatenate([Q, np.zeros((pad, C_OUT), np.float32)], axis=0)
    Cp = np.concatenate([C, np.zeros((pad, C_OUT), np.float32)], axis=0)
    np_pad = np.concatenate([npts, np.ones(pad, np.int64)])

    order = np.argsort(-np_pad, kind="stable")
    ns = np_pad[order]
    sched = tuple(int(ns[N_CORES * TILE_P * i]) for i in range(N_SLOTS))

    # stationaries: S[32g+4j+c, w, m] = A[c, m%64] if j == 2w + m//64
    A16 = A.astype(np.float16)
    S_small = np.zeros((32, 4, 128), np.float16)
    for w in range(4):
        for half in range(2):
            jj = 2 * w + half
            S_small[4 * jj : 4 * jj + 4, w, 64 * half : 64 * half + 64] = A16
    S = np.tile(S_small, (4, 1, 1))  # [128,4,128]

    Vs = Vp[order]
    in_maps = []
    for k in range(N_CORES):
        mp = {"S": S}
        for i, maxN in enumerate(sched):
            G = (maxN + 7) // 8
            c = N_CORES * i + k
            sl = slice(TILE_P * c, TILE_P * (c + 1))
            mp[f"T{i}"] = np.ascontiguousarray(
                Vs[sl][:, : 8 * G, :].transpose(1, 2, 0).reshape(32 * G, TILE_P)
            )
        in_maps.append(mp)
    return in_maps, sched, order, Qp[order], Cp[order]


def _gather(results, sched, order, Qs, Cs):
    smax = np.empty((P_PAD, C_OUT), np.float32)
    for k in range(N_CORES):
        for i in range(N_SLOTS):
            Ok = results[k][f"O{i}"]  # [128, n_planes, 512] fp16
            pm = Ok.max(axis=1)       # [128, 512]
            fold = np.maximum(pm[:C_OUT, :], pm[C_OUT:, :]).astype(np.float32)
            c = N_CORES * i + k
            smax[TILE_P * c : TILE_P * (c + 1)] = fold.T
    out_sorted = np.maximum(np.maximum(smax - Qs, Cs), 0.0)
    out_full = np.empty_like(out_sorted)
    out_full[order] = out_sorted
    return np.ascontiguousarray(out_full[:P_FULL])


def kernel(**inputs):
    from concourse.bass_utils import run_bass_kernel_spmd

    in_maps, sched, order, Qs, Cs = _plan(**inputs)
    if sched not in _CACHE:
        _CACHE[sched] = _build_nc(sched)
    res = run_bass_kernel_spmd(_CACHE[sched], in_maps, list(range(N_CORES)))
    return _gather(res.results, sched, order, Qs, Cs)
